# revision 1
# baseline (speedup 1.0000x reference)
"""Trainium2 Bass kernel for nn_IntensityLoss (bilateral-filter intensity loss).

Math (all window sums use raw r_weights; the 1/25 normalizations cancel):
  A  = sum_t w_t                (25-tap sum, per pixel)
  Bf = sum_t fake_t  w_t ; Cf = sum_t fake_t^2  w_t   (taps = 5x5 shifted copies)
  Bg, Cg  likewise for gamma_hdr
  Bh = sum_t H_t w_t  with  H = hdr_original_im ** (1 - f)   (zero-padded)
  Vx  = max(Cx*A - Bx^2, 0) + eps*A^2        (= A^2 * (var + eps))
  num = K * sqrt(Vg) * (Bh + eps*A)          (K = gray_max / f)
  den = A * sqrt(Vf) + num
  r   = num / den                            (= 1 - std_fake/(std_fake+std_obj))
  out = sum(r * (A-1)) / sum(A-1)            (global over B*H*W pixels)

Sharding: core c handles batch b=c//2, rows [256*(c%2), +256).  Each core pads
to 275 "virtual" rows (11 chunks x 25 rows); pad rows get w=0.04 so A=1,
w_blf=A-1~0 -> no contribution.

Layout: "diagonal stack" [125 partitions = 5 row-shifts x 25 rows, 512 cols].
DVE does the 25 window products per chunk in bf16 2x mode; PE reduces over
taps with sparse selector matmuls accumulating in PSUM; epilogue in fp32.
"""

import sys

sys.path.insert(0, "/opt/trn_rl_repo")

import numpy as np
import ml_dtypes

import concourse.bass as bass
import concourse.bacc as bacc
import concourse.tile as tile
from concourse import mybir
from concourse.bass_utils import run_bass_kernel_spmd

F32 = mybir.dt.float32
BF16 = mybir.dt.bfloat16
AF = mybir.ActivationFunctionType
ALU = mybir.AluOpType
AX = mybir.AxisListType

EPS = 1e-5
H_IMG = 512
W_IMG = 512
B_SZ = 4
N_CORES = 8
RPC = 256          # real rows per core
QR = 25            # rows per chunk
NCH = 11           # chunks per core (275 virtual rows)
VROWS = NCH * QR   # 275
PROWS = 280        # padded image rows staged per core
PCOLS = 516        # padded image cols
WPAD = 0.04        # weight value for virtual-pad rows (A ~= 1)

_CACHE = {}


def _build_nc():
    nc = bacc.Bacc(None)
    wslab = nc.declare_dram_parameter("wslab", [5, VROWS, 5, W_IMG], F32, isOutput=False)
    imf = nc.declare_dram_parameter("imf", [PROWS, PCOLS], F32, isOutput=False)
    img = nc.declare_dram_parameter("img", [PROWS, PCOLS], F32, isOutput=False)
    imh = nc.declare_dram_parameter("imh", [PROWS, PCOLS], F32, isOutput=False)
    hmask = nc.declare_dram_parameter("hmask", [PROWS, 1], F32, isOutput=False)
    gray = nc.declare_dram_parameter("gray", [H_IMG, W_IMG], F32, isOutput=False)
    scal = nc.declare_dram_parameter("scal", [1, 4], F32, isOutput=False)
    stat = nc.declare_dram_parameter("stat", [5, 125, 125], BF16, isOutput=False)
    out = nc.declare_dram_parameter("out", [125, 2], F32, isOutput=True)

    himg = nc.dram_tensor("himg", [PROWS, PCOLS], BF16)

    HW = VROWS * W_IMG  # tap stride in wslab

    with tile.TileContext(nc) as tc:
        with (
            tc.tile_pool(name="singles", bufs=1) as singles,
            tc.tile_pool(name="prep", bufs=2) as prep,
            tc.tile_pool(name="chunk", bufs=3) as chunk,
            tc.tile_pool(name="prod", bufs=2) as prod,
            tc.tile_pool(name="epi", bufs=1) as epi,
            tc.tile_pool(name="psA", bufs=1, space="PSUM") as psum_stats,
            tc.tile_pool(name="psB", bufs=1, space="PSUM") as psum_misc,
        ):
            # ---------- phase 0: scalars, gray max, H image ----------
            ones = singles.tile([1, 128], F32)
            nc.vector.memset(ones[:], 1.0)

            sc = singles.tile([1, 4], F32)
            nc.sync.dma_start(out=sc[:], in_=scal[:])

            # broadcast 1-f and 1/f to all partitions via PE
            f1m_bc = singles.tile([128, 1], F32)
            finv_bc = singles.tile([128, 1], F32)
            ps_bc = psum_misc.tile([128, 1], F32, tag="bc")
            nc.tensor.matmul(ps_bc[:], ones[:], sc[0:1, 0:1], start=True, stop=True)
            nc.scalar.copy(f1m_bc[:], ps_bc[:])
            ps_bc2 = psum_misc.tile([128, 1], F32, tag="bc", name="ps_bc2")
            nc.tensor.matmul(ps_bc2[:], ones[:], sc[0:1, 1:2], start=True, stop=True)
            nc.scalar.copy(finv_bc[:], ps_bc2[:])

            # gray max over the full batch image
            gt = prep.tile([128, 2048], F32)
            nc.sync.dma_start(
                out=gt[:],
                in_=bass.AP(tensor=gray, offset=0, ap=[[2048, 128], [1, 2048]]),
            )
            gm = singles.tile([128, 1], F32)
            nc.vector.tensor_reduce(gm[:], gt[:], axis=AX.X, op=ALU.max)
            gmr = singles.tile([1, 128], F32)
            nc.sync.dma_start(out=gmr[:], in_=gm[:])
            gms = singles.tile([1, 1], F32)
            nc.vector.tensor_reduce(gms[:], gmr[:], axis=AX.X, op=ALU.max)
            gm_bc = singles.tile([128, 1], F32)
            ps_bc3 = psum_misc.tile([128, 1], F32, tag="bc", name="ps_bc3")
            nc.tensor.matmul(ps_bc3[:], ones[:], gms[0:1, 0:1], start=True, stop=True)
            nc.scalar.copy(gm_bc[:], ps_bc3[:])
            k_sb = singles.tile([128, 1], F32)
            nc.vector.tensor_mul(k_sb[:], gm_bc[:], finv_bc[:])

            # H = (hdr ** (1-f)) with zero padding, stored to DRAM in bf16
            row_tiles = [(0, 128), (128, 128), (256, PROWS - 256)]
            for r0, p in row_tiles:
                ht = prep.tile([128, PCOLS], F32, tag="ht")
                nc.sync.dma_start(out=ht[:p, :], in_=imh[r0 : r0 + p, :])
                lt = prep.tile([128, PCOLS], F32, tag="lt")
                nc.scalar.activation(lt[:p, :], ht[:p, :], AF.Ln)
                et = prep.tile([128, PCOLS], BF16, tag="et")
                nc.scalar.activation(et[:p, :], lt[:p, :], AF.Exp, scale=f1m_bc[:p, :])
                hm = prep.tile([128, 1], F32, tag="hm")
                nc.sync.dma_start(out=hm[:p, :], in_=hmask[r0 : r0 + p, :])
                nc.vector.tensor_scalar_mul(et[:p, :], et[:p, :], hm[:p, 0:1])
                nc.vector.memset(et[:p, 0:2], 0.0)
                nc.vector.memset(et[:p, 514:516], 0.0)
                nc.sync.dma_start(out=himg[r0 : r0 + p, :], in_=et[:p, :])

            # stationary selector matrices
            st_all = singles.tile([125, 5, 125], BF16)
            nc.sync.dma_start(
                out=st_all[:],
                in_=bass.AP(
                    tensor=stat,
                    offset=0,
                    ap=[[125, 125], [125 * 125, 5], [1, 125]],
                ),
            )

            # persistent stats accumulators [125, 3 groups, 512]
            stats = {}
            for name in ["A", "Bf", "Cf", "Bg", "Cg", "Bh"]:
                stats[name] = singles.tile([125, 3, 512], F32, tag=f"st{name}", name=f"st{name}")

            # ---------- phase 1: chunks ----------
            ps = {}
            for c in range(NCH):
                s = c % 5
                g = c // 5
                last_s = 4 if g < 2 else 0
                cr0 = c * QR

                wt = chunk.tile([125, 5, 512], BF16, tag="wt")
                nc.gpsimd.dma_start(
                    out=wt[:],
                    in_=bass.AP(
                        tensor=wslab,
                        offset=cr0 * 5 * W_IMG,
                        ap=[[VROWS * 5 * W_IMG, 5], [5 * W_IMG, QR], [1, 5 * W_IMG]],
                    ),
                )

                def rstack(src, shift, tag, cast):
                    wcols = PCOLS - shift
                    t = chunk.tile([125, PCOLS], BF16, tag=tag, name=tag)
                    eng = nc.gpsimd if cast else nc.sync
                    eng.dma_start(
                        out=t[:, 0:wcols],
                        in_=bass.AP(
                            tensor=src,
                            offset=cr0 * PCOLS + shift,
                            ap=[[PCOLS, 5], [PCOLS, QR], [1, wcols]],
                        ),
                    )
                    return t

                rf0 = rstack(imf, 0, "rf0", True)
                rf1 = rstack(imf, 1, "rf1", True)
                rg0 = rstack(img, 0, "rg0", True)
                rg1 = rstack(img, 1, "rg1", True)
                rh0 = rstack(himg, 0, "rh0", False)
                rh1 = rstack(himg, 1, "rh1", False)

                p1f = prod.tile([125, 5, 512], BF16, tag="p1f")
                p2f = prod.tile([125, 5, 512], BF16, tag="p2f")
                p1g = prod.tile([125, 5, 512], BF16, tag="p1g")
                p2g = prod.tile([125, 5, 512], BF16, tag="p2g")
                p1h = prod.tile([125, 5, 512], BF16, tag="p1h")

                for b in range(5):
                    off = b if b % 2 == 0 else b - 1
                    rf = rf0 if b % 2 == 0 else rf1
                    rg = rg0 if b % 2 == 0 else rg1
                    rh = rh0 if b % 2 == 0 else rh1
                    nc.vector.tensor_mul(
                        p1f[:, b, :], rf[:, off : off + 512], wt[:, b, :]
                    )
                    nc.vector.tensor_mul(
                        p2f[:, b, :], p1f[:, b, :], rf[:, off : off + 512]
                    )
                    nc.vector.tensor_mul(
                        p1g[:, b, :], rg[:, off : off + 512], wt[:, b, :]
                    )
                    nc.vector.tensor_mul(
                        p2g[:, b, :], p1g[:, b, :], rg[:, off : off + 512]
                    )
                    nc.vector.tensor_mul(
                        p1h[:, b, :], rh[:, off : off + 512], wt[:, b, :]
                    )

                if s == 0:
                    ps = {
                        name: psum_stats.tile(
                            [125, 512], F32, tag=f"ps{name}", name=f"ps{name}"
                        )
                        for name in ["A", "Bf", "Cf", "Bg", "Cg", "Bh"]
                    }
                movs = {
                    "A": wt,
                    "Bf": p1f,
                    "Cf": p2f,
                    "Bg": p1g,
                    "Cg": p2g,
                    "Bh": p1h,
                }
                for name, mov in movs.items():
                    for b in range(5):
                        nc.tensor.matmul(
                            ps[name][:],
                            st_all[:, s, :],
                            mov[:, b, :],
                            start=(s == 0 and b == 0),
                            stop=(s == last_s and b == 4),
                        )
                if s == last_s:
                    for name in movs:
                        nc.scalar.copy(stats[name][:, g, :], ps[name][:])

            # ---------- phase 2: epilogue (fp32, [125, 1536]) ----------
            # group 2 rows 25..124 hold zeros (never written by real data);
            # they are excluded from the final reduces, and den gets +1e-30 so
            # the reciprocal stays finite there

            def et32(tag):
                return epi.tile([125, 3, 512], F32, tag=tag, name=tag)

            A = stats["A"]
            t1 = et32("t1")
            t2 = et32("t2")
            e2 = et32("e2")
            vf = stats["Cf"]   # overwritten in place (Cf dead after t1)
            vg = stats["Cg"]
            num = stats["Bf"]  # dead after its square is taken
            den = stats["Bg"]

            nc.vector.tensor_mul(e2[:], A[:], A[:])
            nc.vector.tensor_scalar_mul(e2[:], e2[:], EPS)           # eps*A^2
            # vf = sqrt(max(Cf*A - Bf^2, 0) + eps*A^2)
            nc.vector.tensor_mul(t1[:], stats["Cf"][:], A[:])
            nc.vector.tensor_mul(t2[:], stats["Bf"][:], stats["Bf"][:])
            nc.vector.tensor_tensor(vf[:], t1[:], t2[:], op=ALU.subtract)
            nc.vector.tensor_scalar_max(vf[:], vf[:], 0.0)
            nc.vector.tensor_add(vf[:], vf[:], e2[:])
            nc.scalar.activation(vf[:], vf[:], AF.Sqrt)
            # vg likewise
            nc.vector.tensor_mul(t1[:], stats["Cg"][:], A[:])
            nc.vector.tensor_mul(t2[:], stats["Bg"][:], stats["Bg"][:])
            nc.vector.tensor_tensor(vg[:], t1[:], t2[:], op=ALU.subtract)
            nc.vector.tensor_scalar_max(vg[:], vg[:], 0.0)
            nc.vector.tensor_add(vg[:], vg[:], e2[:])
            nc.scalar.activation(vg[:], vg[:], AF.Sqrt)

            # num = K * sqrt(Vg) * (Bh + eps*A)   (overwrites Bf storage)
            nc.vector.tensor_scalar_mul(t1[:], A[:], EPS)
            nc.vector.tensor_add(t1[:], t1[:], stats["Bh"][:])
            # den = A * sqrt(Vf) first (Bg must die before num overwrites Bf? no:
            # num aliases Bf which is already consumed; den aliases Bg, consumed)
            nc.vector.tensor_mul(den[:], A[:], vf[:])
            nc.vector.tensor_mul(num[:], vg[:], t1[:])
            nc.vector.tensor_scalar_mul(num[:], num[:], k_sb[0:125, 0:1])
            nc.vector.tensor_add(den[:], den[:], num[:])
            nc.vector.tensor_scalar_add(den[:], den[:], 1e-30)
            nc.vector.reciprocal(den[:], den[:])
            nc.vector.tensor_mul(num[:], num[:], den[:])             # r
            nc.vector.tensor_scalar_add(t2[:], A[:], -1.0)           # w_blf
            nc.vector.tensor_mul(num[:], num[:], t2[:])              # contrib

            red = epi.tile([125, 2], F32, tag="red")
            redb = epi.tile([125, 2], F32, tag="redb")
            nc.vector.tensor_reduce(red[:, 0:1], num[:, 0:2, :], axis=AX.XY, op=ALU.add)
            nc.vector.tensor_reduce(red[:, 1:2], t2[:, 0:2, :], axis=AX.XY, op=ALU.add)
            nc.vector.tensor_reduce(redb[0:25, 0:1], num[0:25, 2, :], axis=AX.X, op=ALU.add)
            nc.vector.tensor_reduce(redb[0:25, 1:2], t2[0:25, 2, :], axis=AX.X, op=ALU.add)
            nc.vector.tensor_add(red[0:25, :], red[0:25, :], redb[0:25, :])
            nc.sync.dma_start(out=out[:], in_=red[:])

    nc.compile()
    return nc


def _host_inputs(fake, gamma_hdr, hdr_original_im, r_weights, f_factors,
                 hdr_original_gray):
    """Build the 8 per-core input dicts."""
    stat_np = np.zeros((5, 125, 125), dtype=np.float32)
    for s in range(5):
        for a in range(5):
            for q in range(25):
                stat_np[s, a * 25 + q, s * 25 + q] = 1.0
    stat_np = stat_np.astype(ml_dtypes.bfloat16)

    def padimg(x, cval):
        return np.pad(x, ((2, 22), (2, 2)), constant_values=cval).astype(
            np.float32
        )

    in_maps = []
    for c in range(N_CORES):
        b = c // 2
        r0 = (c % 2) * RPC
        slab = np.full((5, 5, VROWS, W_IMG), WPAD, dtype=np.float32)
        slab[:, :, :RPC, :] = r_weights[b, :, r0 : r0 + RPC, :].reshape(
            5, 5, RPC, W_IMG
        )
        slab = np.ascontiguousarray(slab.transpose(0, 2, 1, 3))  # [a, r, b, c]

        pf = padimg(fake[b, 0], 0.0)[r0 : r0 + PROWS]
        pg = padimg(gamma_hdr[b, 0], 0.0)[r0 : r0 + PROWS]
        ph = padimg(hdr_original_im[b, 0], 1.0)[r0 : r0 + PROWS]
        gidx = r0 + np.arange(PROWS)
        hm = ((gidx >= 2) & (gidx <= 513)).astype(np.float32).reshape(PROWS, 1)

        f = float(f_factors[b])
        scal = np.array([[1.0 - f, 1.0 / f, 0.0, 0.0]], dtype=np.float32)

        in_maps.append(
            {
                "wslab": np.ascontiguousarray(slab),
                "imf": np.ascontiguousarray(pf),
                "img": np.ascontiguousarray(pg),
                "imh": np.ascontiguousarray(ph),
                "hmask": hm,
                "gray": np.ascontiguousarray(hdr_original_gray[b, 0]),
                "scal": scal,
                "stat": stat_np,
            }
        )
    return in_maps


def kernel_run(inputs, **spmd_kwargs):
    """Returns (scalar_result, BassKernelResults)."""
    if "nc" not in _CACHE:
        _CACHE["nc"] = _build_nc()
    nc = _CACHE["nc"]
    in_maps = _host_inputs(**inputs)
    res = run_bass_kernel_spmd(nc, in_maps, list(range(N_CORES)), **spmd_kwargs)
    s1 = 0.0
    s2 = 0.0
    for r in res.results:
        o = np.asarray(r["out"], dtype=np.float64)
        s1 += o[:, 0].sum()
        s2 += o[:, 1].sum()
    return np.float32(s1 / s2), res


def kernel(**inputs):
    result, _ = kernel_run(inputs)
    return result

